# revision 35
# baseline (speedup 1.0000x reference)
"""Trainium2 Bass kernel for nn_DepthToTextPromptFuser.

Pipeline (per batch b): depth -> point cloud -> surface normals (unit vectors)
-> single-head attention over N=9216 tokens with head_dim=3 -> per-channel
mean |out| ("dom") -> 4 booleans -> prompt embedding assembly.

Key structural facts exploited:
  * Attention scores s(n,m) = scale * q(x_n).k(x_m) where x are unit normals
    (or zero on the border), so exp(s) restricted to the data manifold
    (S^2 u {0}) x (S^2 u {0}) is numerically low rank.  We build a Nystrom
    factorization on the host from the actual weights:
        exp(scale * q(x).k(y)^T) ~= A(x) @ M @ B(y)
    with P anchor points on the sphere (P=32, escalating to 64/128 if the
    host-side validation over the whole manifold needs it).  This turns the
    N^2 softmax (85M exps per batch) into two thin exp-score matrices (P x N)
    plus tiny matmuls, all computed on device in bf16 (the final outputs only
    depend on the attention through 4 thresholded booleans with ~0.2+ margin,
    and full-pipeline simulations put the bf16+Nystrom error at ~5e-3).
  * Wo / bv / bo fold into the value projection on the host, and the softmax
    denominator is obtained by augmenting values with a ones column.
  * Vertical (partition-axis) stencil shifts come from DMA-shifted copies of
    the depth rows, since compute engines require 32-aligned base partitions.
  * One stationary-weight load per key token tile produces both the anchor
    scores and the value projection; query blocks stack 4-per-PSUM-tile via
    PE column groups so each ACT exp covers 2048 tokens.

Sharding: 8 cores = 2 batches x 4 query chunks of 2304 tokens. Each core
computes its batch's normals + key-side sums (redundantly within a batch,
it is cheap) and the attention outputs for its query chunk, returning
partial |out| sums plus depth-statistics sums. The host reduces partials,
evaluates the 4 thresholds and assembles the (text_emb, text) outputs.
"""

import numpy as np

import concourse.bacc as bacc
import concourse.bass as bass


def _patch_act_tables_once():
    """Make bacc's ACT-table chooser use natural_log_exp_and_others for all of
    Exp/Ln/Copy/Identity/Abs so the kernel needs a single table load."""
    if getattr(bacc, "_ant_act_tables_patched", False):
        return
    orig = bacc.get_activation_tables
    keepset = "natural_log_exp_and_others"
    ours = {
        mybir.ActivationFunctionType.Exp,
        mybir.ActivationFunctionType.Ln,
        mybir.ActivationFunctionType.Copy,
        mybir.ActivationFunctionType.Identity,
        mybir.ActivationFunctionType.Abs,
    }

    def patched(arch):
        t = orig(arch)
        return {
            name: (funcs if name == keepset else set(funcs) - ours)
            for name, funcs in t.items()
        }

    bacc.get_activation_tables = patched
    bacc._ant_act_tables_patched = True
import concourse.tile as tile
from concourse import mybir
from concourse.bass import ds
from concourse.bass_utils import run_bass_kernel_spmd

F32 = mybir.dt.float32
BF16 = mybir.dt.bfloat16
AF = mybir.ActivationFunctionType

H = 96
W = 96
N = H * W                    # 9216 tokens
B = 2
NCHUNK = 4                   # query chunks per batch
CHUNK = N // NCHUNK          # 2304
P = 32                       # Nystrom anchor count (32/64/128 supported)
NBANDS = 128 // P            # query blocks stacked per PSUM tile
SCALE = 1.0 / np.sqrt(3.0)
FX = 500.0
FY = 500.0
DEPTH_SCALE = 1000.0
EMBED_DIM = 24
IN = H - 2                   # 94: interior size for the normal stencil

BASE_IDS = np.array([0, 1, 2, 3, 6, 7, 10])
HORIZ_IDS = np.array([11, 12, 13, 14])
VERT_IDS = np.array([11, 12, 15, 16])

# set by kernel() after each run; test harness reads exec_time_ns from here
LAST_RESULTS = None


def _fib_sphere(n):
    i = np.arange(n, dtype=np.float64) + 0.5
    phi = np.arccos(1.0 - 2.0 * i / n)
    theta = np.pi * (1.0 + 5.0**0.5) * i
    return np.stack(
        [np.cos(theta) * np.sin(phi), np.sin(theta) * np.sin(phi), np.cos(phi)],
        axis=1,
    )


def _host_consts(Wq, bq, Wk, bk, Wv, bv, Wo, bo):
    """Build the Nystrom factorization + folded weights (all f64, cast f32).

    xt row order on device: [ones, x, y, z].
    Value/output column order: [den, v0, v1, v2].
    """
    Wq, bq, Wk, bk, Wv, bv, Wo, bo = [
        np.asarray(a, dtype=np.float64) for a in (Wq, bq, Wk, bk, Wv, bv, Wo, bo)
    ]
    xh = np.vstack([_fib_sphere(P - 1), np.zeros((1, 3))])  # anchors incl. zero token
    qh = xh @ Wq.T + bq                                     # [P,3]
    kh = xh @ Wk.T + bk
    C = np.exp(SCALE * qh @ kh.T)                           # [P,P]
    U, s, Vt = np.linalg.svd(C)

    import ml_dtypes

    bf16 = ml_dtypes.bfloat16
    # validation grid over the whole data manifold (unit sphere + zero token);
    # A/B are rounded to bf16 like the device computes them, M stays f32
    xt = np.vstack([_fib_sphere(797), np.zeros((1, 3))])
    qt = xt @ Wq.T + bq
    kt = xt @ Wk.T + bk
    K_true = np.exp(SCALE * qt @ kt.T)
    A = np.exp(SCALE * qt @ kh.T).astype(bf16).astype(np.float64)
    Bm = np.exp(SCALE * qh @ kt.T).astype(bf16).astype(np.float64)
    # metric: worst-case relative row-sum of |K_hat - K| -- this is what the
    # attention numerator/denominator sums actually see (pointwise max error
    # does NOT control the systematic bias accumulated over 9216 keys)
    true_rowsum = K_true.sum(axis=1)
    M, best_err = None, np.inf
    for rcond in (3e-3, 1e-3, 3e-4, 1e-4, 3e-5, 1e-5, 3e-6, 1e-6):
        cut = s > s[0] * rcond
        Mc = (Vt[cut].T / s[cut]) @ U[:, cut].T
        if np.abs(Mc).max() > 1e4:
            break  # f32 M exactness and device noise both degrade past this
        Kerr = A @ Mc.astype(np.float32).astype(np.float64) @ Bm - K_true
        err = (np.abs(Kerr).sum(axis=1) / true_rowsum).max()
        if err < best_err:
            M, best_err = Mc, err
    _host_consts.last_err = best_err
    f32 = np.float32

    akx = np.empty((4, P), f32)                      # rows: [c, x, y, z]
    akx[0] = (SCALE * (kh @ bq)).astype(f32)
    akx[1:4] = (SCALE * (Wq.T @ kh.T)).astype(f32)   # row d, col p: scale*Wq^T k̂_p

    bqx = np.empty((4, P), f32)
    bqx[0] = (SCALE * (qh @ bk)).astype(f32)
    bqx[1:4] = (SCALE * (Wk.T @ qh.T)).astype(f32)

    wva = np.zeros((4, 4), f32)                      # [xt-row d, value col e]
    wva[0, 0] = 1.0                                  # col 0: ones -> denominator
    wva[0, 1:4] = (Wo @ bv + bo).astype(f32)         # value bias (+bo via den trick)
    wva[1:4, 1:4] = (Wo @ Wv).T.astype(f32)          # value proj with Wo folded in

    uc = np.broadcast_to(
        ((np.arange(W, dtype=np.float64) - W / 2.0) / FX).astype(f32)[None, :], (H, W)
    ).copy()
    vcol = ((np.arange(H, dtype=np.float64) - H / 2.0) / FY).astype(f32)[:, None]

    import ml_dtypes

    bf16 = ml_dtypes.bfloat16
    # bf16 constants packed into one [P, 196] tensor (one DMA):
    # cols 0:64 = M^T, 64:128 = akx (rows 0-3), 128:192 = bqx, 192:196 = wva
    pkb = np.zeros((P, 2 * P + 4), dtype=bf16)
    pkb[0:4, 0:P] = akx.astype(bf16)
    pkb[0:4, P : 2 * P] = bqx.astype(bf16)
    pkb[0:4, 2 * P : 2 * P + 4] = wva.astype(bf16)
    mtf = np.zeros((128, 128), f32)             # [M^T x NBANDS], f32 (bf16 M
    for r in range(NBANDS):                     # rounding is catastrophic for
        mtf[0:P, r * P : (r + 1) * P] = M.astype(f32).T  # truncated pinv M)
    # f32 constants packed with the depth map per-core: cols 0:96 depth,
    # 96:192 uc, 192 vc1 (rows 0:94), 193 vc2 (rows 0:94)
    pkf_const = np.zeros((H, 194), dtype=f32)
    pkf_const[:, 96:192] = uc
    pkf_const[0:IN, 192] = vcol[1 : H - 1, 0]
    pkf_const[0:IN, 193] = vcol[2:H, 0]
    # row 0 = ones (xt homogeneous row), rows 1-3 = zeros (border init)
    onz = np.zeros((4, N), dtype=bf16)
    onz[0] = 1.0
    return {
        "pkb": pkb,
        "mtf": mtf,
        "pkf_const": pkf_const,
        "onesrow": onz,
    }


def _build_program():
    import os

    _patch_act_tables_once()
    nc = bacc.Bacc(None, target_bir_lowering=False)
    pkf_in = nc.dram_tensor("pkf", [H, 194], F32, kind="ExternalInput")
    pkb_in = nc.dram_tensor("pkb", [P, 2 * P + 4], BF16, kind="ExternalInput")
    mtf_in = nc.dram_tensor("mtf", [128, 128], F32, kind="ExternalInput")
    ones_in = nc.dram_tensor("onesrow", [4, N], BF16, kind="ExternalInput")
    res_out = nc.dram_tensor("res", [8, 1], F32, kind="ExternalOutput")

    with tile.TileContext(nc) as tc:
        with (
            tc.tile_pool(name="singles", bufs=1) as singles,
            tc.tile_pool(name="work", bufs=1) as work,
            tc.tile_pool(name="ebpool", bufs=3) as ebpool,
            tc.tile_pool(name="psA", bufs=3, space="PSUM") as psA,
            tc.tile_pool(name="psC", bufs=1, space="PSUM") as psC,
        ):
            # ---- load packed constants & depth on three parallel DMA rings
            # (SP + ACT HWDGE rings, gpsimd SWDGE); row-shifted depth copies
            # because compute engines need 32-aligned partition bases ----
            pkf = work.tile([H, 194], F32, tag="pkf")
            nc.sync.dma_start(out=pkf, in_=pkf_in[:, :])
            dsb1 = work.tile([IN, W], F32, tag="dsb1")      # depth rows 1..94
            nc.scalar.dma_start(out=dsb1, in_=pkf_in[1 : H - 1, 0:W])
            dsb2 = work.tile([IN, W], F32, tag="dsb2")      # depth rows 2..95
            nc.gpsimd.dma_start(out=dsb2, in_=pkf_in[2:H, 0:W])
            pkb = singles.tile([P, 2 * P + 4], BF16, tag="pkb")
            nc.scalar.dma_start(out=pkb, in_=pkb_in[:, :])
            mtf = singles.tile([128, 128], F32, tag="mtf")
            nc.sync.dma_start(out=mtf, in_=mtf_in[:, :])
            dsb = pkf[:, 0:W]
            uc = pkf[:, W : 2 * W]
            vc1 = pkf[0:IN, 192:193]
            vc2 = pkf[0:IN, 193:194]
            mtd = mtf[0:P, :]                    # [M^T x NBANDS], f32
            akx = pkb[0:4, 0:P]
            bvq = pkb[0:4, P : 2 * P + 4]        # [bqx | wva]

            # core id -> query-chunk offset, loaded early so the register is
            # ready long before the A-side matmuls
            pid = nc.partition_id()
            qoff = (pid % NCHUNK) * CHUNK

            # ---- depth statistics: sum and sum of squares over the image ----
            st96 = work.tile([H, 2], F32, tag="st96")
            nc.vector.tensor_reduce(
                st96[:, 0:1], dsb, axis=mybir.AxisListType.X, op=mybir.AluOpType.add
            )
            dsq = work.tile([H, W], F32, tag="dsq")
            nc.gpsimd.tensor_mul(dsq, dsb, dsb)
            nc.vector.tensor_reduce(
                st96[:, 1:2], dsq, axis=mybir.AxisListType.X, op=mybir.AluOpType.add
            )
            ones96 = singles.tile([H, 1], F32, tag="ones96")
            nc.vector.memset(ones96, 1.0)
            ps_stat = psC.tile([2, 1], F32, tag="csmall")
            nc.tensor.matmul(ps_stat, st96, ones96, start=True, stop=True)
            stat_sb = work.tile([2, 1], F32, tag="stat_sb")
            nc.vector.tensor_copy(stat_sb, ps_stat)

            # ---- point clouds [h, ch, w] (ch 0=X, 1=Y, 2=Z) for image rows
            # 1..94 (center) and 2..95 (down) ----
            pcc = work.tile([IN, 3, W], F32, tag="pcc")
            nc.vector.tensor_scalar_mul(pcc[:, 2, :], dsb1, 1.0 / DEPTH_SCALE)
            nc.vector.tensor_mul(pcc[:, 0, :], pcc[:, 2, :], uc[0:IN, :])
            nc.vector.tensor_scalar_mul(pcc[:, 1, :], pcc[:, 2, :], vc1)
            pcd = work.tile([IN, 3, W], F32, tag="pcd")
            nc.gpsimd.tensor_scalar_mul(pcd[:, 2, :], dsb2, 1.0 / DEPTH_SCALE)
            nc.gpsimd.tensor_mul(pcd[:, 0, :], pcd[:, 2, :], uc[0:IN, :])
            nc.gpsimd.tensor_scalar_mul(pcd[:, 1, :], pcd[:, 2, :], vc2)

            # ---- stencil diffs, channels duplicated so the cross product can
            # use contiguous channel windows ----
            e1d = work.tile([IN, 5, IN], F32, tag="e1d")
            nc.vector.tensor_sub(
                e1d[:, 0:3, :], pcc[:, :, 2:W], pcc[:, :, 1 : W - 1]
            )
            nc.gpsimd.tensor_copy(e1d[:, 3:5, :], e1d[:, 0:2, :])
            e2d = work.tile([IN, 5, IN], F32, tag="e2d")
            nc.vector.tensor_sub(
                e2d[:, 0:3, :], pcd[:, :, 1 : W - 1], pcc[:, :, 1 : W - 1]
            )
            nc.gpsimd.tensor_copy(e2d[:, 3:5, :], e2d[:, 0:2, :])

            # ---- cross product cr[ch] = e1[ch+1]*e2[ch+2] - e1[ch+2]*e2[ch+1] ----
            cr = work.tile([IN, 3, IN], F32, tag="cr")
            m1 = work.tile([IN, 3, IN], F32, tag="m1")
            nc.vector.tensor_mul(m1, e1d[:, 1:4, :], e2d[:, 2:5, :])
            nc.vector.tensor_mul(cr, e1d[:, 2:5, :], e2d[:, 1:4, :])
            nc.vector.tensor_sub(cr, m1, cr)

            # ---- 1/norm via exp(-0.5*ln(s2)) (Ln+Exp live in one ACT table set) ----
            sq = work.tile([IN, 3, IN], F32, tag="sq")
            nc.vector.tensor_mul(sq, cr, cr)
            s2 = work.tile([IN, IN], F32, tag="s2")
            nc.vector.tensor_reduce(
                s2,
                sq[:, :, :].rearrange("h c w -> h w c"),
                axis=mybir.AxisListType.X,
                op=mybir.AluOpType.add,
            )
            nc.vector.tensor_scalar_max(s2, s2, 1e-24)
            nc.scalar.activation(s2, s2, AF.Ln)
            nc.scalar.activation(s2, s2, AF.Exp, scale=-0.5)   # s2 <- 1/sqrt(s2)

            # ---- normals interior [94, 3, 94] ----
            crn = work.tile([IN, 3, IN], BF16, tag="crn")
            for ch in range(3):
                nc.vector.tensor_mul(crn[:, ch, :], cr[:, ch, :], s2)

            # ---- assemble xt [4, N] (rows [ones, x, y, z]) directly in SBUF:
            # row 0 ones + rows 1:4 zeroed via broadcast DMA, then the normals
            # interior written with a strided row pattern (borders stay 0) ----
            xt = work.tile([4, N], BF16, tag="xt")
            nc.sync.dma_start(out=xt, in_=ones_in[:, :])
            xt_int = xt[:, :].rearrange("d (h w) -> d h w", h=H)
            int_eng = [nc.sync, nc.scalar, nc.gpsimd]
            for ch in range(3):
                int_eng[ch].dma_start(
                    out=xt_int[1 + ch : 2 + ch, 1 : H - 1, 1 : W - 1],
                    in_=crn[:, ch, :],
                )

            # ---- key side: one stationary load per token tile computes both
            # the anchor scores (cols 0:32) and the value projection (32:36);
            # then exp the scores and accumulate G = sum_m expB[m,:] (x) vt[m,:] ----
            NT = N // 128  # 72 token tiles
            BVW = P + 4    # output columns per token tile
            GPT = 512 // BVW   # token tiles per PSUM bank group
            vt = work.tile([128, NT, 4], BF16, tag="vt")
            ps_g = psC.tile([P, 4], F32, tag="csmall")
            for g0 in range(0, NT, GPT):
                gn = min(GPT, NT - g0)
                pb = psA.tile([128, GPT * BVW], F32, tag="workB")
                for j in range(gn):
                    nc.tensor.matmul(
                        pb[:, BVW * j : BVW * (j + 1)],
                        xt[:, 128 * (g0 + j) : 128 * (g0 + j + 1)],
                        bvq,
                        start=True,
                        stop=True,
                    )
                pbv = pb[:, :].rearrange("m (j c) -> m j c", c=BVW)
                nc.vector.tensor_copy(
                    vt[:, g0 : g0 + gn, :], pbv[:, 0:gn, P : P + 4]
                )
                eb = ebpool.tile([128, GPT, P], BF16, tag="eb")
                nc.scalar.activation(eb[:, 0:gn, :], pbv[:, 0:gn, 0:P], AF.Exp)
                for j in range(gn):
                    t = g0 + j
                    nc.tensor.matmul(
                        ps_g,
                        eb[:, j, :],
                        vt[:, t, :],
                        start=(t == 0),
                        stop=(t == NT - 1),
                    )

            # ---- query-side scores + exp: four 512-token blocks stack on
            # partition bands 32j via PE column groups (one exp per 2048) ----
            qt_sizes = []
            o = 0
            while o < CHUNK:
                qt_sizes.append((o, min(512, CHUNK - o)))
                o += 512
            n_slots = (len(qt_sizes) + NBANDS - 1) // NBANDS
            ea = work.tile([128, n_slots * 512, 1], BF16, tag="ea")
            ea_slot = {}
            pa = None
            for i, (o, sz) in enumerate(qt_sizes):
                band = (i % NBANDS) * P
                slot = (i // NBANDS) * 512
                ea_slot[o] = (band, slot)
                if i % NBANDS == 0:
                    pa = psA.tile([128, 512], F32, tag="workA")
                nc.tensor.matmul(
                    pa[band : band + P, :sz],
                    akx,
                    xt[:, ds(qoff + o, sz)],
                    start=True,
                    stop=True,
                    tile_position=(0, band),
                )
                if i % NBANDS == NBANDS - 1 or i == len(qt_sizes) - 1:
                    hi = band + P
                    nc.scalar.activation(
                        ea[0:hi, slot : slot + sz, 0], pa[0:hi, :sz], AF.Exp
                    )
            g_sb = work.tile([P, 4], F32, tag="g_sb")
            nc.scalar.copy(g_sb, ps_g)

            # ---- H = M @ G  (lhsT = M^T) ----
            ps_h = psC.tile([128, 4], F32, tag="cbig")
            nc.tensor.matmul(ps_h, mtd, g_sb, start=True, stop=True)
            h_sb = work.tile([128, 4], BF16, tag="h_sb")
            nc.scalar.copy(h_sb, ps_h)

            # ---- out4 = H^T A^T: rows [den, v0, v1, v2] ----
            num_sb = work.tile([4, CHUNK], F32, tag="num_sb")
            rec = work.tile([1, CHUNK], BF16, tag="rec")
            for i, (o, sz) in enumerate(qt_sizes):
                band, slot = ea_slot[o]
                po = psA.tile([4, 512], F32, tag="workA")
                nc.tensor.matmul(
                    po[:, :sz],
                    h_sb[band : band + P, :],
                    ea[band : band + P, slot : slot + sz, 0],
                    start=True,
                    stop=True,
                    tile_position=(band, 0),
                )
                nc.scalar.copy(num_sb[:, o : o + sz], po[:, :sz])
                with nc.allow_low_precision("dom threshold margin is ~0.3"):
                    nc.vector.reciprocal(rec[:, o : o + sz], num_sb[0:1, o : o + sz])

            # ---- per-token divide by denominator (row 0), |.|, sum over tokens ----
            ones14 = singles.tile([1, 4], BF16, tag="ones14")
            nc.vector.memset(ones14, 1.0)
            t2 = work.tile([4, CHUNK], F32, tag="t2")
            parts = work.tile([4, len(qt_sizes)], F32, tag="parts")
            for i, (o, sz) in enumerate(qt_sizes):
                prb = psA.tile([4, 512], F32, tag="workA")
                nc.tensor.matmul(
                    prb[:, :sz], ones14, rec[:, o : o + sz], start=True, stop=True
                )
                nc.vector.tensor_mul(
                    t2[:, o : o + sz], num_sb[:, o : o + sz], prb[:, :sz]
                )
                nc.scalar.activation(
                    t2[:, o : o + sz],
                    t2[:, o : o + sz],
                    AF.Abs,
                    accum_out=parts[:, i : i + 1],
                )
            acc4 = work.tile([4, 1], F32, tag="acc4")
            nc.vector.tensor_reduce(
                acc4,
                parts,
                axis=mybir.AxisListType.X,
                op=mybir.AluOpType.add,
            )

            # ---- outputs: rows 1-3 |out| partial sums, 4 depth sum, 5 sumsq ----
            nc.sync.dma_start(out=res_out[0:4, :], in_=acc4)
            nc.sync.dma_start(out=res_out[4:6, :], in_=stat_sb)
    nc.finalize()
    return nc


_PROGRAM_CACHE = {}


def _get_program():
    if P not in _PROGRAM_CACHE:
        _PROGRAM_CACHE[P] = _build_program()
    return _PROGRAM_CACHE[P]


def _set_p(pval):
    global P, NBANDS
    P = pval
    NBANDS = 128 // P


def kernel(depth_map, emb_table, Wq, Wk, Wv, bq, bk, bv, Wo, bo):
    global LAST_RESULTS
    import os

    depth_map = np.asarray(depth_map, dtype=np.float32)
    emb_table = np.asarray(emb_table, dtype=np.float32)
    consts = None
    for pval in (32, 64, 128):
        _set_p(pval)
        consts = _host_consts(Wq, bq, Wk, bk, Wv, bv, Wo, bo)
        if _host_consts.last_err < 5e-3:
            break

    d = depth_map.mean(axis=1)  # [B, H, W]; C=1 so this is just a squeeze

    def _pack_pkf(const_part, db):
        pk = const_part.copy()
        pk[:, 0:W] = db
        return pk

    nc = _get_program()
    in_maps = []
    for c in range(8):
        b = c // NCHUNK
        in_maps.append(
            {
                "pkf": _pack_pkf(consts["pkf_const"], d[b]),
                "pkb": consts["pkb"],
                "mtf": consts["mtf"],
                "onesrow": consts["onesrow"],
            }
        )

    trace = bool(int(os.environ.get("KERNEL_TRACE", "0")))
    results = run_bass_kernel_spmd(
        nc,
        in_maps,
        core_ids=list(range(8)),
        trace=trace,
        trace_cores=list(range(8)) if trace else None,
    )
    LAST_RESULTS = results
    res = [r["res"] for r in results.results]  # each [8, 1]

    f32 = np.float32
    text_emb = np.zeros((B, EMBED_DIM), f32)
    for b in range(B):
        cores = [res[b * NCHUNK + j] for j in range(NCHUNK)]
        dom = sum(r[1:4, 0] for r in cores).astype(f32) / f32(N)
        dsum = cores[0][4, 0]
        dsumsq = cores[0][5, 0]
        avg = dsum / f32(N)
        var = (dsumsq - dsum * dsum / f32(N)) / f32(N - 1)

        is_close = bool(avg < 5.0)
        is_complex = bool(var > 0.5)
        horiz = bool(dom[1] > 0.5)
        vert = bool((dom[0] > 0.5) or (dom[2] > 0.5))

        base_sum = emb_table[BASE_IDS].sum(axis=0, dtype=f32)
        horiz_sum = emb_table[HORIZ_IDS].sum(axis=0, dtype=f32)
        vert_sum = emb_table[VERT_IDS].sum(axis=0, dtype=f32)
        dist_emb = emb_table[4] if is_close else emb_table[5]
        comp_emb = emb_table[8] if is_complex else emb_table[9]
        hf = f32(1.0 if horiz else 0.0)
        vf = f32(1.0 if vert else 0.0)
        total = base_sum + dist_emb + comp_emb + hf * horiz_sum + vf * vert_sum
        count = f32(9.0) + f32(4.0) * hf + f32(4.0) * vf
        text_emb[b] = total / count

    text = np.ascontiguousarray(
        np.broadcast_to(text_emb[:, :, None, None], (B, EMBED_DIM, H, W))
    ).astype(f32)
    return (text_emb, text)


# revision 38
# speedup vs baseline: 1.0264x; 1.0264x over previous
"""Trainium2 Bass kernel for nn_DepthToTextPromptFuser.

Pipeline (per batch b): depth -> point cloud -> surface normals (unit vectors)
-> single-head attention over N=9216 tokens with head_dim=3 -> per-channel
mean |out| ("dom") -> 4 booleans -> prompt embedding assembly.

Key structural facts exploited:
  * Attention scores s(n,m) = scale * q(x_n).k(x_m) where x are unit normals
    (or zero on the border), so exp(s) restricted to the data manifold
    (S^2 u {0}) x (S^2 u {0}) is numerically low rank.  We build a Nystrom
    factorization on the host from the actual weights:
        exp(scale * q(x).k(y)^T) ~= A(x) @ M @ B(y)
    with P anchor points on the sphere (P=32, escalating to 64/128 if the
    host-side validation over the whole manifold needs it).  This turns the
    N^2 softmax (85M exps per batch) into two thin exp-score matrices (P x N)
    plus tiny matmuls, all computed on device in bf16 (the final outputs only
    depend on the attention through 4 thresholded booleans with ~0.2+ margin,
    and full-pipeline simulations put the bf16+Nystrom error at ~5e-3).
  * Wo / bv / bo fold into the value projection on the host, and the softmax
    denominator is obtained by augmenting values with a ones column.
  * Vertical (partition-axis) stencil shifts come from DMA-shifted copies of
    the depth rows, since compute engines require 32-aligned base partitions.
  * One stationary-weight load per key token tile produces both the anchor
    scores and the value projection; query blocks stack 4-per-PSUM-tile via
    PE column groups so each ACT exp covers 2048 tokens.

Sharding: 8 cores = 2 batches x 4 query chunks of 2304 tokens. Each core
computes its batch's normals + key-side sums (redundantly within a batch,
it is cheap) and the attention outputs for its query chunk, returning
partial |out| sums plus depth-statistics sums. The host reduces partials,
evaluates the 4 thresholds and assembles the (text_emb, text) outputs.
"""

import numpy as np

import concourse.bacc as bacc
import concourse.bass as bass


def _patch_act_tables_once():
    """Make bacc's ACT-table chooser use natural_log_exp_and_others for all of
    Exp/Ln/Copy/Identity/Abs so the kernel needs a single table load."""
    if getattr(bacc, "_ant_act_tables_patched", False):
        return
    orig = bacc.get_activation_tables
    keepset = "natural_log_exp_and_others"
    ours = {
        mybir.ActivationFunctionType.Exp,
        mybir.ActivationFunctionType.Ln,
        mybir.ActivationFunctionType.Copy,
        mybir.ActivationFunctionType.Identity,
        mybir.ActivationFunctionType.Abs,
    }

    def patched(arch):
        t = orig(arch)
        return {
            name: (funcs if name == keepset else set(funcs) - ours)
            for name, funcs in t.items()
        }

    bacc.get_activation_tables = patched
    bacc._ant_act_tables_patched = True
import concourse.tile as tile
from concourse import mybir
from concourse.bass import ds
from concourse.bass_utils import run_bass_kernel_spmd

F32 = mybir.dt.float32
BF16 = mybir.dt.bfloat16
AF = mybir.ActivationFunctionType

H = 96
W = 96
N = H * W                    # 9216 tokens
B = 2
NCHUNK = 4                   # query chunks per batch
CHUNK = N // NCHUNK          # 2304
P = 32                       # Nystrom anchor count (32/64/128 supported)
NBANDS = 128 // P            # query blocks stacked per PSUM tile
SCALE = 1.0 / np.sqrt(3.0)
FX = 500.0
FY = 500.0
DEPTH_SCALE = 1000.0
EMBED_DIM = 24
IN = H - 2                   # 94: interior size for the normal stencil

BASE_IDS = np.array([0, 1, 2, 3, 6, 7, 10])
HORIZ_IDS = np.array([11, 12, 13, 14])
VERT_IDS = np.array([11, 12, 15, 16])

# set by kernel() after each run; test harness reads exec_time_ns from here
LAST_RESULTS = None


def _fib_sphere(n):
    i = np.arange(n, dtype=np.float64) + 0.5
    phi = np.arccos(1.0 - 2.0 * i / n)
    theta = np.pi * (1.0 + 5.0**0.5) * i
    return np.stack(
        [np.cos(theta) * np.sin(phi), np.sin(theta) * np.sin(phi), np.cos(phi)],
        axis=1,
    )


def _host_consts(Wq, bq, Wk, bk, Wv, bv, Wo, bo):
    """Build the Nystrom factorization + folded weights (all f64, cast f32).

    xt row order on device: [ones, x, y, z].
    Value/output column order: [den, v0, v1, v2].
    """
    Wq, bq, Wk, bk, Wv, bv, Wo, bo = [
        np.asarray(a, dtype=np.float64) for a in (Wq, bq, Wk, bk, Wv, bv, Wo, bo)
    ]
    xh = np.vstack([_fib_sphere(P - 1), np.zeros((1, 3))])  # anchors incl. zero token
    qh = xh @ Wq.T + bq                                     # [P,3]
    kh = xh @ Wk.T + bk
    C = np.exp(SCALE * qh @ kh.T)                           # [P,P]
    U, s, Vt = np.linalg.svd(C)

    import ml_dtypes

    bf16 = ml_dtypes.bfloat16
    # validation grid over the whole data manifold (unit sphere + zero token);
    # A/B are rounded to bf16 like the device computes them, M stays f32
    xt = np.vstack([_fib_sphere(797), np.zeros((1, 3))])
    qt = xt @ Wq.T + bq
    kt = xt @ Wk.T + bk
    K_true = np.exp(SCALE * qt @ kt.T)
    A = np.exp(SCALE * qt @ kh.T).astype(bf16).astype(np.float64)
    Bm = np.exp(SCALE * qh @ kt.T).astype(bf16).astype(np.float64)
    # metric: worst-case relative row-sum of |K_hat - K| -- this is what the
    # attention numerator/denominator sums actually see (pointwise max error
    # does NOT control the systematic bias accumulated over 9216 keys)
    true_rowsum = K_true.sum(axis=1)
    M, best_err = None, np.inf
    for rcond in (3e-3, 1e-3, 3e-4, 1e-4, 3e-5, 1e-5, 3e-6, 1e-6):
        cut = s > s[0] * rcond
        Mc = (Vt[cut].T / s[cut]) @ U[:, cut].T
        if np.abs(Mc).max() > 1e4:
            break  # f32 M exactness and device noise both degrade past this
        Kerr = A @ Mc.astype(np.float32).astype(np.float64) @ Bm - K_true
        err = (np.abs(Kerr).sum(axis=1) / true_rowsum).max()
        if err < best_err:
            M, best_err = Mc, err
    _host_consts.last_err = best_err
    f32 = np.float32

    akx = np.empty((4, P), f32)                      # rows: [c, x, y, z]
    akx[0] = (SCALE * (kh @ bq)).astype(f32)
    akx[1:4] = (SCALE * (Wq.T @ kh.T)).astype(f32)   # row d, col p: scale*Wq^T k̂_p

    bqx = np.empty((4, P), f32)
    bqx[0] = (SCALE * (qh @ bk)).astype(f32)
    bqx[1:4] = (SCALE * (Wk.T @ qh.T)).astype(f32)

    wva = np.zeros((4, 4), f32)                      # [xt-row d, value col e]
    wva[0, 0] = 1.0                                  # col 0: ones -> denominator
    wva[0, 1:4] = (Wo @ bv + bo).astype(f32)         # value bias (+bo via den trick)
    wva[1:4, 1:4] = (Wo @ Wv).T.astype(f32)          # value proj with Wo folded in

    uc = np.broadcast_to(
        ((np.arange(W, dtype=np.float64) - W / 2.0) / FX).astype(f32)[None, :], (H, W)
    ).copy()
    vcol = ((np.arange(H, dtype=np.float64) - H / 2.0) / FY).astype(f32)[:, None]

    import ml_dtypes

    bf16 = ml_dtypes.bfloat16
    # bf16 constants packed into one [P, 196] tensor (one DMA):
    # cols 0:64 = M^T, 64:128 = akx (rows 0-3), 128:192 = bqx, 192:196 = wva
    pkb = np.zeros((P, 2 * P + 4), dtype=bf16)
    pkb[0:4, 0:P] = akx.astype(bf16)
    pkb[0:4, P : 2 * P] = bqx.astype(bf16)
    pkb[0:4, 2 * P : 2 * P + 4] = wva.astype(bf16)
    mtf = np.zeros((128, 128), f32)             # [M^T x NBANDS], f32 (bf16 M
    for r in range(NBANDS):                     # rounding is catastrophic for
        mtf[0:P, r * P : (r + 1) * P] = M.astype(f32).T  # truncated pinv M)
    # f32 constants packed with the depth map per-core: cols 0:96 depth,
    # 96:192 uc, 192 vc1 (rows 0:94), 193 vc2 (rows 0:94)
    pkf_const = np.zeros((H, 194), dtype=f32)
    pkf_const[:, 96:192] = uc
    pkf_const[0:IN, 192] = vcol[1 : H - 1, 0]
    pkf_const[0:IN, 193] = vcol[2:H, 0]
    # row 0 = ones (xt homogeneous row), rows 1-3 = zeros (border init)
    onz = np.zeros((4, N), dtype=bf16)
    onz[0] = 1.0
    return {
        "pkb": pkb,
        "mtf": mtf,
        "pkf_const": pkf_const,
        "onesrow": onz,
    }


def _build_program():
    import os

    _patch_act_tables_once()
    nc = bacc.Bacc(None, target_bir_lowering=False)
    pkf_in = nc.dram_tensor("pkf", [H, 194], F32, kind="ExternalInput")
    pkb_in = nc.dram_tensor("pkb", [P, 2 * P + 4], BF16, kind="ExternalInput")
    mtf_in = nc.dram_tensor("mtf", [128, 128], F32, kind="ExternalInput")
    ones_in = nc.dram_tensor("onesrow", [4, N], BF16, kind="ExternalInput")
    res_out = nc.dram_tensor("res", [8, 1], F32, kind="ExternalOutput")

    with tile.TileContext(nc) as tc:
        with (
            tc.tile_pool(name="singles", bufs=1) as singles,
            tc.tile_pool(name="work", bufs=1) as work,
            tc.tile_pool(name="ebpool", bufs=6) as ebpool,
            tc.tile_pool(name="psA", bufs=3, space="PSUM") as psA,
            tc.tile_pool(name="psC", bufs=1, space="PSUM") as psC,
        ):
            # ---- load packed constants & depth on three parallel DMA rings
            # (SP + ACT HWDGE rings, gpsimd SWDGE); row-shifted depth copies
            # because compute engines need 32-aligned partition bases ----
            pkf = work.tile([H, 194], F32, tag="pkf")
            nc.sync.dma_start(out=pkf, in_=pkf_in[:, :])
            dsb1 = work.tile([IN, W], F32, tag="dsb1")      # depth rows 1..94
            nc.scalar.dma_start(out=dsb1, in_=pkf_in[1 : H - 1, 0:W])
            dsb2 = work.tile([IN, W], F32, tag="dsb2")      # depth rows 2..95
            nc.gpsimd.dma_start(out=dsb2, in_=pkf_in[2:H, 0:W])
            pkb = singles.tile([P, 2 * P + 4], BF16, tag="pkb")
            nc.scalar.dma_start(out=pkb, in_=pkb_in[:, :])
            mtf = singles.tile([128, 128], F32, tag="mtf")
            nc.sync.dma_start(out=mtf, in_=mtf_in[:, :])
            dsb = pkf[:, 0:W]
            uc = pkf[:, W : 2 * W]
            vc1 = pkf[0:IN, 192:193]
            vc2 = pkf[0:IN, 193:194]
            mtd = mtf[0:P, :]                    # [M^T x NBANDS], f32
            akx = pkb[0:4, 0:P]
            bvq = pkb[0:4, P : 2 * P + 4]        # [bqx | wva]

            # core id -> query-chunk offset, loaded early so the register is
            # ready long before the A-side matmuls
            pid = nc.partition_id()
            qoff = (pid % NCHUNK) * CHUNK

            # ---- depth statistics: sum and sum of squares over the image ----
            st96 = work.tile([H, 2], F32, tag="st96")
            nc.vector.tensor_reduce(
                st96[:, 0:1], dsb, axis=mybir.AxisListType.X, op=mybir.AluOpType.add
            )
            dsq = work.tile([H, W], F32, tag="dsq")
            nc.gpsimd.tensor_mul(dsq, dsb, dsb)
            nc.vector.tensor_reduce(
                st96[:, 1:2], dsq, axis=mybir.AxisListType.X, op=mybir.AluOpType.add
            )
            ones96 = singles.tile([H, 1], F32, tag="ones96")
            nc.vector.memset(ones96, 1.0)
            ps_stat = psC.tile([2, 1], F32, tag="csmall")
            nc.tensor.matmul(ps_stat, st96, ones96, start=True, stop=True)
            stat_sb = work.tile([2, 1], F32, tag="stat_sb")
            nc.vector.tensor_copy(stat_sb, ps_stat)

            # ---- point clouds [h, ch, w] (ch 0=X, 1=Y, 2=Z) for image rows
            # 1..94 (center) and 2..95 (down) ----
            pcc = work.tile([IN, 3, W], F32, tag="pcc")
            nc.vector.tensor_scalar_mul(pcc[:, 2, :], dsb1, 1.0 / DEPTH_SCALE)
            nc.vector.tensor_mul(pcc[:, 0, :], pcc[:, 2, :], uc[0:IN, :])
            nc.vector.tensor_scalar_mul(pcc[:, 1, :], pcc[:, 2, :], vc1)
            pcd = work.tile([IN, 3, W], F32, tag="pcd")
            nc.gpsimd.tensor_scalar_mul(pcd[:, 2, :], dsb2, 1.0 / DEPTH_SCALE)
            nc.gpsimd.tensor_mul(pcd[:, 0, :], pcd[:, 2, :], uc[0:IN, :])
            nc.gpsimd.tensor_scalar_mul(pcd[:, 1, :], pcd[:, 2, :], vc2)

            # ---- stencil diffs, channels duplicated so the cross product can
            # use contiguous channel windows ----
            e1d = work.tile([IN, 5, IN], F32, tag="e1d")
            nc.vector.tensor_sub(
                e1d[:, 0:3, :], pcc[:, :, 2:W], pcc[:, :, 1 : W - 1]
            )
            nc.gpsimd.tensor_copy(e1d[:, 3:5, :], e1d[:, 0:2, :])
            e2d = work.tile([IN, 5, IN], F32, tag="e2d")
            nc.vector.tensor_sub(
                e2d[:, 0:3, :], pcd[:, :, 1 : W - 1], pcc[:, :, 1 : W - 1]
            )
            nc.gpsimd.tensor_copy(e2d[:, 3:5, :], e2d[:, 0:2, :])

            # ---- cross product cr[ch] = e1[ch+1]*e2[ch+2] - e1[ch+2]*e2[ch+1] ----
            cr = work.tile([IN, 3, IN], F32, tag="cr")
            m1 = work.tile([IN, 3, IN], F32, tag="m1")
            nc.vector.tensor_mul(m1, e1d[:, 1:4, :], e2d[:, 2:5, :])
            nc.vector.tensor_mul(cr, e1d[:, 2:5, :], e2d[:, 1:4, :])
            nc.vector.tensor_sub(cr, m1, cr)

            # ---- 1/norm via exp(-0.5*ln(s2)) (Ln+Exp live in one ACT table set) ----
            sq = work.tile([IN, 3, IN], F32, tag="sq")
            nc.vector.tensor_mul(sq, cr, cr)
            s2 = work.tile([IN, IN], F32, tag="s2")
            nc.vector.tensor_reduce(
                s2,
                sq[:, :, :].rearrange("h c w -> h w c"),
                axis=mybir.AxisListType.X,
                op=mybir.AluOpType.add,
            )
            nc.vector.tensor_scalar_max(s2, s2, 1e-24)
            nc.scalar.activation(s2, s2, AF.Ln)
            nc.scalar.activation(s2, s2, AF.Exp, scale=-0.5)   # s2 <- 1/sqrt(s2)

            # ---- normals interior [94, 3, 94] ----
            crn = work.tile([IN, 3, IN], BF16, tag="crn")
            for ch in range(3):
                nc.vector.tensor_mul(crn[:, ch, :], cr[:, ch, :], s2)

            # ---- assemble xt [4, N] (rows [ones, x, y, z]) directly in SBUF:
            # row 0 ones + rows 1:4 zeroed via broadcast DMA, then the normals
            # interior written with a strided row pattern (borders stay 0) ----
            xt = work.tile([4, N], BF16, tag="xt")
            nc.sync.dma_start(out=xt, in_=ones_in[:, :])
            xt_int = xt[:, :].rearrange("d (h w) -> d h w", h=H)
            int_eng = [nc.sync, nc.scalar, nc.gpsimd]
            for ch in range(3):
                int_eng[ch].dma_start(
                    out=xt_int[1 + ch : 2 + ch, 1 : H - 1, 1 : W - 1],
                    in_=crn[:, ch, :],
                )

            # ---- key side: one stationary load per token tile computes both
            # the anchor scores (cols 0:32) and the value projection (32:36);
            # then exp the scores and accumulate G = sum_m expB[m,:] (x) vt[m,:] ----
            NT = N // 128  # 72 token tiles
            BVW = P + 4    # output columns per token tile
            GPT = 512 // BVW   # token tiles per PSUM bank group
            vt = work.tile([128, NT, 4], BF16, tag="vt")
            ps_g = psC.tile([P, 4], F32, tag="csmall")
            for g0 in range(0, NT, GPT):
                gn = min(GPT, NT - g0)
                pb = psA.tile([128, GPT * BVW], F32, tag="workB")
                for j in range(gn):
                    nc.tensor.matmul(
                        pb[:, BVW * j : BVW * (j + 1)],
                        xt[:, 128 * (g0 + j) : 128 * (g0 + j + 1)],
                        bvq,
                        start=True,
                        stop=True,
                    )
                pbv = pb[:, :].rearrange("m (j c) -> m j c", c=BVW)
                nc.vector.tensor_copy(
                    vt[:, g0 : g0 + gn, :], pbv[:, 0:gn, P : P + 4]
                )
                eb = ebpool.tile([128, GPT, P], BF16, tag="eb")
                nc.scalar.activation(eb[:, 0:gn, :], pbv[:, 0:gn, 0:P], AF.Exp)
                for j in range(gn):
                    t = g0 + j
                    nc.tensor.matmul(
                        ps_g,
                        eb[:, j, :],
                        vt[:, t, :],
                        start=(t == 0),
                        stop=(t == NT - 1),
                    )

            # ---- query-side scores + exp: four 512-token blocks stack on
            # partition bands 32j via PE column groups (one exp per 2048) ----
            qt_sizes = []
            o = 0
            while o < CHUNK:
                qt_sizes.append((o, min(512, CHUNK - o)))
                o += 512
            n_slots = (len(qt_sizes) + NBANDS - 1) // NBANDS
            ea = work.tile([128, n_slots * 512, 1], BF16, tag="ea")
            ea_slot = {}
            pa = None
            for i, (o, sz) in enumerate(qt_sizes):
                band = (i % NBANDS) * P
                slot = (i // NBANDS) * 512
                ea_slot[o] = (band, slot)
                if i % NBANDS == 0:
                    pa = psA.tile([128, 512], F32, tag="workA")
                nc.tensor.matmul(
                    pa[band : band + P, :sz],
                    akx,
                    xt[:, ds(qoff + o, sz)],
                    start=True,
                    stop=True,
                    tile_position=(0, band),
                )
                if i % NBANDS == NBANDS - 1 or i == len(qt_sizes) - 1:
                    hi = band + P
                    nc.scalar.activation(
                        ea[0:hi, slot : slot + sz, 0], pa[0:hi, :sz], AF.Exp
                    )
            g_sb = work.tile([P, 4], F32, tag="g_sb")
            nc.vector.tensor_copy(g_sb, ps_g)

            # ---- H = M @ G  (lhsT = M^T) ----
            ps_h = psC.tile([128, 4], F32, tag="cbig")
            nc.tensor.matmul(ps_h, mtd, g_sb, start=True, stop=True)
            h_sb = work.tile([128, 4], BF16, tag="h_sb")
            nc.vector.tensor_copy(h_sb, ps_h)

            # ---- out4 = H^T A^T: rows [den, v0, v1, v2] ----
            num_sb = work.tile([4, CHUNK], F32, tag="num_sb")
            rec = work.tile([1, CHUNK], BF16, tag="rec")
            for i, (o, sz) in enumerate(qt_sizes):
                band, slot = ea_slot[o]
                po = psA.tile([4, 512], F32, tag="workA")
                nc.tensor.matmul(
                    po[:, :sz],
                    h_sb[band : band + P, :],
                    ea[band : band + P, slot : slot + sz, 0],
                    start=True,
                    stop=True,
                    tile_position=(band, 0),
                )
                nc.scalar.copy(num_sb[:, o : o + sz], po[:, :sz])
                with nc.allow_low_precision("dom threshold margin is ~0.3"):
                    nc.vector.reciprocal(rec[:, o : o + sz], num_sb[0:1, o : o + sz])

            # ---- per-token divide by denominator (row 0), |.|, sum over tokens ----
            ones14 = singles.tile([1, 4], BF16, tag="ones14")
            nc.vector.memset(ones14, 1.0)
            t2 = work.tile([4, CHUNK], F32, tag="t2")
            parts = work.tile([4, len(qt_sizes)], F32, tag="parts")
            for i, (o, sz) in enumerate(qt_sizes):
                prb = psA.tile([4, 512], F32, tag="workA")
                nc.tensor.matmul(
                    prb[:, :sz], ones14, rec[:, o : o + sz], start=True, stop=True
                )
                nc.vector.tensor_mul(
                    t2[:, o : o + sz], num_sb[:, o : o + sz], prb[:, :sz]
                )
                nc.scalar.activation(
                    t2[:, o : o + sz],
                    t2[:, o : o + sz],
                    AF.Abs,
                    accum_out=parts[:, i : i + 1],
                )
            acc4 = work.tile([4, 1], F32, tag="acc4")
            nc.vector.tensor_reduce(
                acc4,
                parts,
                axis=mybir.AxisListType.X,
                op=mybir.AluOpType.add,
            )

            # ---- outputs: rows 1-3 |out| partial sums, 4 depth sum, 5 sumsq ----
            nc.sync.dma_start(out=res_out[0:4, :], in_=acc4)
            nc.sync.dma_start(out=res_out[4:6, :], in_=stat_sb)
    nc.finalize()
    return nc


_PROGRAM_CACHE = {}


def _get_program():
    if P not in _PROGRAM_CACHE:
        _PROGRAM_CACHE[P] = _build_program()
    return _PROGRAM_CACHE[P]


def _set_p(pval):
    global P, NBANDS
    P = pval
    NBANDS = 128 // P


def kernel(depth_map, emb_table, Wq, Wk, Wv, bq, bk, bv, Wo, bo):
    global LAST_RESULTS
    import os

    depth_map = np.asarray(depth_map, dtype=np.float32)
    emb_table = np.asarray(emb_table, dtype=np.float32)
    consts = None
    for pval in (32, 64, 128):
        _set_p(pval)
        consts = _host_consts(Wq, bq, Wk, bk, Wv, bv, Wo, bo)
        if _host_consts.last_err < 5e-3:
            break

    d = depth_map.mean(axis=1)  # [B, H, W]; C=1 so this is just a squeeze

    def _pack_pkf(const_part, db):
        pk = const_part.copy()
        pk[:, 0:W] = db
        return pk

    nc = _get_program()
    in_maps = []
    for c in range(8):
        b = c // NCHUNK
        in_maps.append(
            {
                "pkf": _pack_pkf(consts["pkf_const"], d[b]),
                "pkb": consts["pkb"],
                "mtf": consts["mtf"],
                "onesrow": consts["onesrow"],
            }
        )

    trace = bool(int(os.environ.get("KERNEL_TRACE", "0")))
    results = run_bass_kernel_spmd(
        nc,
        in_maps,
        core_ids=list(range(8)),
        trace=trace,
        trace_cores=list(range(8)) if trace else None,
    )
    LAST_RESULTS = results
    res = [r["res"] for r in results.results]  # each [8, 1]

    f32 = np.float32
    text_emb = np.zeros((B, EMBED_DIM), f32)
    for b in range(B):
        cores = [res[b * NCHUNK + j] for j in range(NCHUNK)]
        dom = sum(r[1:4, 0] for r in cores).astype(f32) / f32(N)
        dsum = cores[0][4, 0]
        dsumsq = cores[0][5, 0]
        avg = dsum / f32(N)
        var = (dsumsq - dsum * dsum / f32(N)) / f32(N - 1)

        is_close = bool(avg < 5.0)
        is_complex = bool(var > 0.5)
        horiz = bool(dom[1] > 0.5)
        vert = bool((dom[0] > 0.5) or (dom[2] > 0.5))

        base_sum = emb_table[BASE_IDS].sum(axis=0, dtype=f32)
        horiz_sum = emb_table[HORIZ_IDS].sum(axis=0, dtype=f32)
        vert_sum = emb_table[VERT_IDS].sum(axis=0, dtype=f32)
        dist_emb = emb_table[4] if is_close else emb_table[5]
        comp_emb = emb_table[8] if is_complex else emb_table[9]
        hf = f32(1.0 if horiz else 0.0)
        vf = f32(1.0 if vert else 0.0)
        total = base_sum + dist_emb + comp_emb + hf * horiz_sum + vf * vert_sum
        count = f32(9.0) + f32(4.0) * hf + f32(4.0) * vf
        text_emb[b] = total / count

    text = np.ascontiguousarray(
        np.broadcast_to(text_emb[:, :, None, None], (B, EMBED_DIM, H, W))
    ).astype(f32)
    return (text_emb, text)
